# revision 17
# baseline (speedup 1.0000x reference)
"""Trainium2 Bass kernel for Cafe_RNN decode (26-neighbor argmin solidification).

Full inputs -> shard i-axis across 8 NeuronCores (16 planes each + 1-plane
halo) -> per-core slab program (see build()) -> gather to full output.

The per-core program bit-replicates the fp32 reference:
  d_n = ((ex-rx)^2 + (ey-ry)^2) + (ez-rz)^2  per 26-neighborhood shift,
  argmin with first-index-wins ties via strict < updates in shift order,
  plus the pointwise state/field/euler selection logic.

v2 redesign vs the original baseline:
  - E slabs live in [j, i, k, c] layout and are built by ONE strided DMA
    straight from DRAM (the per-plane ACT rearrange copies are gone).
    j+-1 shifted slabs Em/Ep are single full-range partition-shifted
    SBUF->SBUF DMAs of E0.
  - Boundary handling: all slab pad locations hold exact 0.0. An
    out-of-range neighbor then evaluates to d == ||r||^2 bit-exactly,
    which TIES the pen3-seeded zero-candidate best (strict < keeps the
    seed, whose payload is the same euler = 0). pen3 = 0 on boundary
    rows/cols injects that seed; interior seed is +1e30 (never wins).
  - The 26-shift loop is software-pipelined: iteration i issues
    distance-chain work for shift i, the compare for shift i-2, and the
    predicated euler update for shift i-4, so the DVE never waits on the
    ACT-square / GPSIMD-add chain. Steady-state DVE work per shift is
    subtract + is_lt + copy_predicated; bd's running min lives on GPSIMD.

Engine split (fp32 bit-exactness of ACT Square / GPSIMD add vs DVE
verified on HW by the original baseline): DVE subs/compares/predicated
selects, ACT squares and copies, GPSIMD adds/mins and memsets.
"""
import json
import sys
from contextlib import ExitStack

if '/opt/trn_rl_repo' not in sys.path:
    sys.path.insert(0, '/opt/trn_rl_repo')

import numpy as np

import concourse.bass as bass
import concourse.tile as tile
from concourse import mybir

F32 = mybir.dt.float32
U8 = mybir.dt.uint8
OP = mybir.AluOpType
AF = mybir.ActivationFunctionType
BIG = 1e30

SHIFTS = [(di, dj, dk)
          for di in (-1, 0, 1) for dj in (-1, 0, 1) for dk in (-1, 0, 1)
          if not (di == 0 and dj == 0 and dk == 0)]


# ---------------------------------------------------------------------------
# This container's walrus build rejects instructions carrying more than one
# semaphore wait ("Too many sync wait commands"). Tile's wait-assignment
# attaches several. Split the extras onto NoOps inserted just before.
_shim_installed = False


def _split_waits(bir_bytes):
    m = json.loads(bir_bytes)
    ctr = 0
    changed = False
    for fn in m.get("functions", []):
        for blk in fn.get("blocks", []):
            insts = blk.get("instructions")
            if not insts:
                continue
            out = []
            for inst in insts:
                si = inst.get("sync_info")
                waits = (si or {}).get("on_wait") or []
                if len(waits) > 1:
                    changed = True
                    for w in waits[:-1]:
                        ctr += 1
                        out.append({
                            "debug": inst.get("debug", 0),
                            "engine": inst["engine"],
                            "ins": [],
                            "name": f"waitsplit_{ctr}_{inst['name']}",
                            "opcode": "NoOp",
                            "outs": [],
                            "sync_info": {"on_update": [], "on_wait": [w]},
                        })
                    si["on_wait"] = [waits[-1]]
                out.append(inst)
            blk["instructions"] = out
    return json.dumps(m).encode() if changed else bir_bytes


def _install_shim():
    global _shim_installed
    if _shim_installed:
        return
    _shim_installed = True
    import concourse.bass2jax as bass2jax
    import concourse.bass_utils as bass_utils
    orig = getattr(bass_utils.compile_bir_kernel, "__wrapped__",
                   bass_utils.compile_bir_kernel)

    def patched(bir_json, tmpdir, neff_name="file.neff"):
        if isinstance(bir_json, str):
            bir_json = bir_json.encode()
        return orig(_split_waits(bir_json), tmpdir, neff_name)

    bass_utils.compile_bir_kernel = patched
    bass2jax.compile_bir_kernel = patched


# ---------------------------------------------------------------------------
def build(NP=16, NJ=128, NK=128, NI=4, repeat=1, bd_update="sign"):
    assert NP % NI == 0
    KW = NK + 2
    NPH = NP + 2
    nc = bass.Bass("TRN2", target_bir_lowering=False, debug=False, num_devices=8)

    xs = nc.declare_dram_parameter("xs", [NPH, NJ, NK * 5], F32, isOutput=False)
    os_ = nc.declare_dram_parameter("os", [NP, NJ, NK * 8], F32, isOutput=False)
    y = nc.declare_dram_parameter("y", [NP, NJ, NK * 5], F32, isOutput=True)

    sh = [NJ, NI, NK]
    sh3 = [NJ, 3, NI, NK]
    shm = [NJ, 1, NI, NK]

    with tile.TileContext(nc) as tc, ExitStack() as ctx:
        const = ctx.enter_context(tc.tile_pool(name="const", bufs=1))
        persist = ctx.enter_context(tc.tile_pool(name="persist", bufs=1))
        och = ctx.enter_context(tc.tile_pool(name="och", bufs=2))
        ych = ctx.enter_context(tc.tile_pool(name="ych", bufs=2))
        bestp = ctx.enter_context(tc.tile_pool(name="best", bufs=2))
        tap = ctx.enter_context(tc.tile_pool(name="tap", bufs=3))
        dfp = ctx.enter_context(tc.tile_pool(name="dfp", bufs=2))
        ddp = ctx.enter_context(tc.tile_pool(name="ddp", bufs=3))
        mkp = ctx.enter_context(tc.tile_pool(name="mkp", bufs=3))
        ep = ctx.enter_context(tc.tile_pool(name="ep", bufs=1))
        rcp = ctx.enter_context(tc.tile_pool(name="rcp", bufs=1))

        c_neg1 = const.tile([NJ, 1], F32)
        nc.vector.memset(c_neg1[:, :], -1.0)
        c_one = const.tile([NJ, 1], F32)
        nc.vector.memset(c_one[:, :], 1.0)
        c_two = const.tile([NJ, 1], F32)
        nc.vector.memset(c_two[:, :], 2.0)

        # pen3[j, k]: 0 on grid boundary rows/cols (injects the zero-neighbor
        # candidate as the seeded best there), BIG in the interior.
        pen3 = const.tile([NJ, 1, NK], F32)
        nc.vector.memset(pen3[:], BIG)
        nc.vector.memset(pen3[0:1, :, :], 0.0)
        nc.sync.dma_start(pen3[NJ - 1:NJ, :, :], pen3[0:1, :, :])
        nc.vector.memset(pen3[:, :, 0:1], 0.0)
        nc.vector.memset(pen3[:, :, NK - 1:NK], 0.0)

        # --- E slabs: [NJ, 3, NPH, KW] c-major so every neighbor view has
        # a stride-1 inner k axis (strided inner reads cost ~3x on the DVE).
        E0 = persist.tile([NJ, 3, NPH, KW], F32)
        Em = persist.tile([NJ, 3, NPH, KW], F32)
        Ep = persist.tile([NJ, 3, NPH, KW], F32)
        # zero k-pad columns, then fill the body from staged planes
        nc.vector.memset(E0[:, :, :, 0:1], 0.0)
        nc.vector.memset(E0[:, :, :, KW - 1:KW], 0.0)
        # Stage x planes contiguously (full-BW DMA), extract euler channels
        # into E0 and state0 masks with compute-engine strided copies.
        st_nz = persist.tile([NJ, NP, NK], F32)    # state0 != 0
        st_le1 = persist.tile([NJ, NP, NK], U8)    # state0 <= 1
        CH = 3
        for p0 in range(0, NPH, CH):
            S = och.tile([NJ, CH, NK * 5], F32, tag="o")
            nc.sync.dma_start(
                S[:, :, :], xs[p0:p0 + CH, :, :].rearrange("p j m -> j p m"))
            Sv = S[:, :, :].rearrange("j p (k c) -> j c p k", c=5)
            nc.scalar.copy(E0[:, :, p0:p0 + CH, 1:NK + 1], Sv[:, 1:4])
            a = max(p0, 1)
            b = min(p0 + CH, NP + 1)
            if a < b:
                nc.vector.tensor_scalar(st_le1[:, a - 1:b - 1, :],
                                        Sv[:, 0, a - p0:b - p0, :],
                                        1.5, None, op0=OP.is_le)
                nc.vector.tensor_scalar(st_nz[:, a - 1:b - 1, :],
                                        Sv[:, 0, a - p0:b - p0, :],
                                        0.5, None, op0=OP.is_ge)
        # j+-1 shifted slabs: partition-shifted copies; out-of-range row = 0
        E0f = E0[:, :, :, :].rearrange("j c i k -> j (c i k)")
        Emf = Em[:, :, :, :].rearrange("j c i k -> j (c i k)")
        Epf = Ep[:, :, :, :].rearrange("j c i k -> j (c i k)")
        nc.gpsimd.memset(Em[0:1, :, :, :], 0.0)
        nc.sync.dma_start(Emf[1:NJ], E0f[0:NJ - 1])
        # compute-engine APs must start at partition 0/32/64/96: zero the
        # last quadrant, then overwrite rows 96..126 with the shifted copy.
        nc.gpsimd.memset(Ep[96:NJ, :, :, :], 0.0)
        nc.sync.dma_start(Epf[0:NJ - 1], E0f[1:NJ])
        EJ = {-1: Em, 0: E0, 1: Ep}

        def e_view(c0, di, dj, dk):
            # [NJ, 3, NI, NK] neighbor view of the dj slab (k stride 1)
            return EJ[dj][:, :, 1 + c0 + di:1 + c0 + di + NI,
                          1 + dk:1 + dk + NK]

        for _rep in range(repeat):
          for c0 in range(0, NP, NI):
            O = och.tile([NJ, NI, NK * 8], F32, tag="o")
            nc.sync.dma_start(
                O[:, :, :], os_[c0:c0 + NI, :, :].rearrange("i j k -> j i k"))
            Ov = O[:, :, :].rearrange("j i (k c) -> j c i k", c=8)
            l0, l1, l2, l3 = (Ov[:, q] for q in range(4))
            f = Ov[:, 7]
            rcb = rcp.tile(sh3, F32, tag="rc")
            nc.scalar.copy(rcb[:], Ov[:, 4:7])
            rc = rcb[:]

            # ---- seed: best = ||r||^2 + pen3, payload euler = 0
            tas = tap.tile(sh3, F32, tag="ta")
            nc.scalar.square(tas[:], rc)
            dds = ddp.tile(sh, F32, tag="dd")
            nc.gpsimd.tensor_tensor(dds[:], tas[:, 0], tas[:, 1], op=OP.add)
            nc.gpsimd.tensor_tensor(dds[:], dds[:], tas[:, 2], op=OP.add)
            bd = bestp.tile(sh, F32, tag="bd")
            nc.gpsimd.tensor_tensor(
                bd[:], dds[:], pen3[:, :, :].broadcast_to(sh), op=OP.add)
            bca = bestp.tile(sh3, F32, tag="bca")
            nc.gpsimd.memset(bca[:], 0.0)

            # ---- software-pipelined shift loop
            NSH = len(SHIFTS)
            dd_t = [None] * NSH
            mk_t = [None] * NSH

            def stage_a(n):
                di, dj, dk = SHIFTS[n]
                ta = tap.tile(sh3, F32, tag="ta")
                nc.vector.tensor_tensor(ta[:], e_view(c0, di, dj, dk), rc,
                                        op=OP.subtract)
                nc.scalar.square(ta[:], ta[:])
                dd = ddp.tile(sh, F32, tag="dd")
                nc.gpsimd.tensor_tensor(dd[:], ta[:, 0], ta[:, 1], op=OP.add)
                nc.gpsimd.tensor_tensor(dd[:], dd[:], ta[:, 2], op=OP.add)
                dd_t[n] = dd

            def stage_b(n):
                # mask = u8(Sign(bd - dd)): 1 iff dd < bd (f32->u8 clamps
                # negatives to 0; Sign(0) = 0 keeps strict-less tie rule).
                # Keeps the per-shift compare off the DVE.
                mk = mkp.tile(shm, U8, tag="mk")
                if bd_update == "sign":
                    df = dfp.tile(sh, F32, tag="df")
                    nc.gpsimd.tensor_tensor(df[:], bd[:], dd_t[n][:],
                                            op=OP.subtract)
                    nc.scalar.activation(mk[:, 0], df[:], AF.Sign)
                    nc.vector.tensor_tensor(bd[:], bd[:], dd_t[n][:],
                                            op=OP.min)
                else:
                    nc.vector.tensor_tensor(mk[:, 0], dd_t[n][:], bd[:],
                                            op=OP.is_lt)
                    nc.vector.tensor_tensor(bd[:], bd[:], dd_t[n][:],
                                            op=OP.min)
                mk_t[n] = mk

            def stage_c(n):
                di, dj, dk = SHIFTS[n]
                nc.vector.copy_predicated(
                    bca[:], mk_t[n][:, :, :, :].broadcast_to(sh3),
                    e_view(c0, di, dj, dk))

            for i in range(NSH + 4):
                if i < NSH:
                    stage_a(i)
                if 2 <= i < NSH + 2:
                    stage_b(i - 2)
                if i >= 4:
                    stage_c(i - 4)

            # ---- pointwise epilogue
            Y = ych.tile([NJ, NI, NK * 5], F32, tag="y")
            Yv = Y[:, :, :].rearrange("j i (k c) -> j c i k", c=5)
            y0, y4 = Yv[:, 0], Yv[:, 4]
            y123 = Yv[:, 1:4]

            t1 = ep.tile(sh, F32, tag="t1")
            t3 = ep.tile(sh, F32, tag="t3")
            s1 = ep.tile(sh, F32, tag="s1")
            s2 = ep.tile(sh, F32, tag="s2")
            sc = ep.tile(sh, F32, tag="s1")  # reuses s1 slot (s1 dead)
            u_sw = ep.tile(sh, U8, tag="u_sw")
            u_le1 = ep.tile(shm, U8, tag="u_le1")
            u_eq2 = ep.tile(sh, U8, tag="u_eq2")
            u_fs = ep.tile(shm, U8, tag="u_fs")

            # state1 = argmax4 (first-max-wins), zeroed where state0 == 0
            nc.vector.tensor_tensor(y0, l1, l0, op=OP.is_gt)
            nc.vector.tensor_tensor(t1[:], l0, l1, op=OP.max)
            nc.vector.tensor_tensor(s2[:], l3, l2, op=OP.is_gt)
            nc.vector.tensor_tensor(t3[:], l2, l3, op=OP.max)
            nc.scalar.activation(s1[:], s2[:], AF.Identity,
                                 bias=c_two[:, :], scale=1.0)
            nc.vector.tensor_tensor(u_sw[:], t3[:], t1[:], op=OP.is_gt)
            nc.vector.copy_predicated(y0, u_sw[:], s1[:])
            nc.gpsimd.tensor_tensor(y0, y0, st_nz[:, c0:c0 + NI, :],
                                    op=OP.mult)

            # masks from final state1
            nc.gpsimd.tensor_scalar(u_le1[:, 0], y0, 1.5, None, op0=OP.is_le)
            nc.gpsimd.tensor_scalar(u_eq2[:], y0, 2.0, None, op0=OP.is_equal)
            nc.gpsimd.tensor_tensor(sc[:], y0, st_le1[:, c0:c0 + NI, :],
                                    op=OP.mult)
            nc.gpsimd.tensor_scalar(u_fs[:, 0], sc[:], 1.5, None,
                                    op0=OP.is_ge)

            # field1: 1.0 default (state==3), clip for ==2, -1 for <=1
            nc.vector.tensor_scalar(s2[:], f, 0.0, 0.92, op0=OP.max,
                                    op1=OP.min)
            nc.gpsimd.memset(y4, 1.0)
            nc.vector.copy_predicated(y4, u_eq2[:], s2[:])
            nc.vector.copy_predicated(y4, u_le1[:, 0],
                                      c_neg1[:, :].broadcast_to(sh))

            # euler1: passthrough, then -1 where state1<=1, then snap where
            # the cell just solidified
            nc.scalar.copy(y123, e_view(c0, 0, 0, 0))
            nc.vector.copy_predicated(
                y123, u_le1[:].broadcast_to(sh3),
                c_neg1[:, :].broadcast_to(sh3))
            nc.vector.copy_predicated(y123, u_fs[:].broadcast_to(sh3), bca[:])

            nc.sync.dma_start(
                y[c0:c0 + NI, :, :].rearrange("i j k -> j i k"), Y[:, :, :])
    return nc


_NC = None


def _get_nc():
    global _NC
    if _NC is None:
        _install_shim()
        _NC = build(NP=16, NJ=128, NK=128, NI=4)
    return _NC


def _run(x, out, trace=False):
    from concourse.bass_utils import run_bass_kernel_spmd
    D, NP, NC_ = 128, 16, 8
    x = np.ascontiguousarray(np.asarray(x, dtype=np.float32))
    out = np.ascontiguousarray(np.asarray(out, dtype=np.float32))
    xg = x.reshape(D, D, D * 5)
    og = out.reshape(D, D, D * 8)
    in_maps = []
    for c in range(NC_):
        xs = np.zeros((NP + 2, D, D * 5), np.float32)
        lo = c * NP - 1
        glo, ghi = max(lo, 0), min(c * NP + NP + 1, D)
        xs[glo - lo:ghi - lo] = xg[glo:ghi]
        in_maps.append({"xs": xs,
                        "os": np.ascontiguousarray(og[c * NP:(c + 1) * NP])})
    res = run_bass_kernel_spmd(_get_nc(), in_maps,
                               core_ids=list(range(NC_)), trace=trace)
    yfull = np.concatenate([res.results[c]["y"] for c in range(NC_)], axis=0)
    return yfull.reshape(1, D, D, D, 5), res


def kernel(x, out):
    return _run(x, out)[0]
